# revision 3
# baseline (speedup 1.0000x reference)
"""GCN 2-layer forward on 8 Trainium2 NeuronCores (Bass/Tile).

Strategy (node-sharded, edges bucketed by target window):
  deg/norm:  norm[e] = dinv[row]*dinv[col] is separable -> fold dinv[row] into
             the transformed feature table (row scale) and dinv[col] into the
             output window rows (col scale). Scatter matrices become BINARY
             one-hot -> precomputed on host as fp8, streamed sequentially.
  Phase A:   every core redundantly computes XW1S = (x @ W1) * dinv[:,None]
             (bf16 table in local DRAM) - avoids an AllGather of 51MB.
  Phase B:   per 128-node output window: ~35 chunks of 128 edges; per chunk an
             indirect-DMA gather of xw rows [128,256] and a PE matmul
             psum += S_c^T @ M_c with S_c fp8 one-hot [128 edges,128 nodes].
             Epilogue: h = relu(dinv[col]*psum + b1) -> bf16.
  Phase C:   hw = (h @ W2pad) * dinv[:,None] via DMA-transpose loads of h.
  Phase D:   AllGather hw shards -> full HW2S table [Npad, 64] bf16.
  Phase E:   same aggregation with the SAME S/idx data, then log_softmax.

kernel(**inputs) takes full unsharded inputs, returns full [N, 40] output.
"""
import sys
sys.path.insert(0, "/opt/trn_rl_repo")

import numpy as np
import ml_dtypes

import concourse.bass as bass
import concourse.mybir as mybir
import concourse.tile as tile
from concourse import bacc

BF16 = mybir.dt.bfloat16
FP8 = mybir.dt.float8e4
F32 = mybir.dt.float32
I32 = mybir.dt.int32

N_CORES = 8
P = 128          # partitions / window size / chunk size

_RUN_CACHE = {}


# ----------------------------------------------------------------- host side

def _preprocess(x, edge_index, W1, b1, W2, b2):
    """Build the static layout + per-core input arrays."""
    N, F_in = x.shape
    H = W1.shape[1]
    C = W2.shape[1]
    Cpad = 64
    assert C <= Cpad

    shard = -(-N // (N_CORES * P)) * P          # per-core node count, 128-mult
    Npad = shard * N_CORES
    n_win = shard // P                           # windows per core
    n_tiles = Npad // P                          # global node tiles

    row = np.asarray(edge_index[0], np.int64)
    col = np.asarray(edge_index[1], np.int64)
    # self loops
    loops = np.arange(N, dtype=np.int64)
    row = np.concatenate([row, loops])
    col = np.concatenate([col, loops])

    deg = np.bincount(col, minlength=Npad).astype(np.float64)
    deg[N:] = 1.0
    dinv = (1.0 / np.sqrt(deg)).astype(np.float32)

    # sort edges by col -> per (core, window) buckets; inside: sort by row
    order = np.lexsort((row, col))
    row = row[order]
    col = col[order]
    wid = col // P                               # global window id 0..n_tiles-1
    # counts per (global window)
    wcounts = np.bincount(wid, minlength=n_tiles)
    wstart = np.zeros(n_tiles + 1, np.int64)
    np.cumsum(wcounts, out=wstart[1:])

    # per-core-local window w: chunk count = ceil(max_k count / P)
    cnt2d = wcounts.reshape(N_CORES, n_win)      # [core, w]
    chunks_w = np.maximum(1, -(-cnt2d.max(axis=0) // P))   # [n_win]
    slots_w = chunks_w * P
    slot_off = np.zeros(n_win + 1, np.int64)
    np.cumsum(slots_w, out=slot_off[1:])
    tot_slots = int(slot_off[-1])
    tot_chunks = int(chunks_w.sum())

    # per-core idx / S arrays
    idx_all = np.zeros((N_CORES, P, tot_chunks), np.int32)      # row per slot
    s_all = np.zeros((N_CORES, P, tot_chunks * P), ml_dtypes.float8_e4m3)
    one = ml_dtypes.float8_e4m3(1.0)
    for k in range(N_CORES):
        for w in range(n_win):
            g = k * n_win + w
            s, e = wstart[g], wstart[g + 1]
            r_w = row[s:e]
            c_w = (col[s:e] - g * P).astype(np.int64)   # 0..127
            cbase = int(slot_off[w] // P)
            nslot = int(slots_w[w])
            # slot i -> (p=i%P, chunk=cbase+i//P)
            n_real = len(r_w)
            pp = np.arange(n_real) % P
            cc = cbase + np.arange(n_real) // P
            idx_all[k, pp, cc] = r_w
            s_all[k, pp, cc * P + c_w] = one
            # pad slots keep idx 0 / S 0

    # dinv tiled layouts
    dinv_t = dinv[: n_tiles * P].reshape(n_tiles, P).T.copy()   # [P, n_tiles]
    xT = np.zeros((F_in, Npad), ml_dtypes.bfloat16)
    xT[:, :N] = np.asarray(x, np.float32).T.astype(ml_dtypes.bfloat16)

    W1b = np.asarray(W1, np.float32).astype(ml_dtypes.bfloat16)         # [F_in, H]
    W2p = np.zeros((H, Cpad), ml_dtypes.bfloat16)
    W2p[:, :C] = np.asarray(W2, np.float32).astype(ml_dtypes.bfloat16)
    b1t = np.tile(np.asarray(b1, np.float32)[None, :], (P, 1))          # [P, H]
    b2t = np.zeros((P, Cpad), np.float32)
    b2t[:, :C] = np.asarray(b2, np.float32)[None, :]

    layout = dict(
        N=N, F_in=F_in, H=H, C=C, Cpad=Cpad, shard=shard, Npad=Npad,
        n_win=n_win, n_tiles=n_tiles,
        chunks_w=[int(v) for v in chunks_w],
        chunk_off=np.concatenate([[0], np.cumsum(chunks_w)]).astype(int).tolist(),
        tot_chunks=tot_chunks, tot_slots=tot_slots,
    )

    in_maps = []
    for k in range(N_CORES):
        in_maps.append({
            "xT": xT,
            "W1": W1b,
            "W2p": W2p,
            "b1t": b1t,
            "b2t": b2t,
            "dinv_t": dinv_t,
            "dinv_own": dinv_t[:, k * n_win:(k + 1) * n_win].copy(),
            "idx": idx_all[k],
            "soh": s_all[k],
        })
    return layout, in_maps


# --------------------------------------------------------------- bass program

def _build(L):
    Np, H, F_in, Cpad = L["Npad"], L["H"], L["F_in"], L["Cpad"]
    n_win, n_tiles = L["n_win"], L["n_tiles"]
    shard = L["shard"]
    chunks_w, chunk_off = L["chunks_w"], L["chunk_off"]
    tot_chunks = L["tot_chunks"]
    KT1 = F_in // P          # k-tiles for x@W1
    KT2 = H // P             # k-tiles for h@W2

    nc = bacc.Bacc("TRN2", target_bir_lowering=False, debug=False,
                   enable_asserts=True, num_devices=N_CORES)

    xT = nc.dram_tensor("xT", [F_in, Np], BF16, kind="ExternalInput")
    W1 = nc.dram_tensor("W1", [F_in, H], BF16, kind="ExternalInput")
    W2p = nc.dram_tensor("W2p", [H, Cpad], BF16, kind="ExternalInput")
    b1t = nc.dram_tensor("b1t", [P, H], F32, kind="ExternalInput")
    b2t = nc.dram_tensor("b2t", [P, Cpad], F32, kind="ExternalInput")
    dinv_t = nc.dram_tensor("dinv_t", [P, n_tiles], F32, kind="ExternalInput")
    dinv_own = nc.dram_tensor("dinv_own", [P, n_win], F32, kind="ExternalInput")
    idx = nc.dram_tensor("idx", [P, tot_chunks], I32, kind="ExternalInput")
    soh = nc.dram_tensor("soh", [P, tot_chunks * P], FP8, kind="ExternalInput")
    out = nc.dram_tensor("out", [shard, Cpad], F32, kind="ExternalOutput")

    xw1s = nc.dram_tensor("xw1s", [Np, H], BF16, kind="Internal")
    h_loc = nc.dram_tensor("h_loc", [shard, H], BF16, kind="Internal")
    hw_loc = nc.dram_tensor("hw_loc", [shard, Cpad], BF16, kind="Internal")
    hw2s = nc.dram_tensor("hw2s", [Np, Cpad], BF16, kind="Internal",
                          addr_space="Shared")

    NBLK = 16                # node tiles per xT slab
    n_blk = n_tiles // NBLK
    assert n_tiles % NBLK == 0

    with tile.TileContext(nc) as tc:
        with (
            tc.tile_pool(name="const", bufs=1) as constp,
            tc.tile_pool(name="slab", bufs=2) as slabp,
            tc.tile_pool(name="stage", bufs=3) as stagep,
            tc.tile_pool(name="gth", bufs=8) as gthp,
            tc.tile_pool(name="sld", bufs=2) as sldp,
            tc.tile_pool(name="epi", bufs=3) as epip,
            tc.tile_pool(name="psA", bufs=2, space="PSUM") as psA,
            tc.tile_pool(name="psB", bufs=2, space="PSUM") as psB,
            tc.tile_pool(name="psC", bufs=2, space="PSUM") as psC,
            tc.tile_pool(name="psE", bufs=2, space="PSUM") as psE,
        ):
            # resident constants
            w1_t = constp.tile([P, KT1, H], BF16)
            nc.sync.dma_start(w1_t[:], W1[:].rearrange("(k p) h -> p k h", p=P))
            w2_t = constp.tile([P, KT2, Cpad], BF16)
            nc.sync.dma_start(w2_t[:], W2p[:].rearrange("(k p) c -> p k c", p=P))
            b1_t = constp.tile([P, H], F32)
            nc.sync.dma_start(b1_t[:], b1t[:])
            b2_t = constp.tile([P, Cpad], F32)
            nc.sync.dma_start(b2_t[:], b2t[:])
            dinv_tt = constp.tile([P, n_tiles], F32)
            nc.sync.dma_start(dinv_tt[:], dinv_t[:])
            dinv_ot = constp.tile([P, n_win], F32)
            nc.sync.dma_start(dinv_ot[:], dinv_own[:])
            idx_t = constp.tile([P, tot_chunks], I32)
            nc.sync.dma_start(idx_t[:], idx[:])

            # ---------------- phase A: XW1S table (all nodes, redundant)
            for blk in range(n_blk):
                xs = slabp.tile([P, KT1, NBLK * P], BF16, tag="xslab")
                nc.sync.dma_start(
                    xs[:],
                    xT[:, blk * NBLK * P:(blk + 1) * NBLK * P]
                    .rearrange("(k p) n -> p k n", p=P))
                for t in range(NBLK):
                    g = blk * NBLK + t
                    ps = psA.tile([P, H], F32, space="PSUM")
                    for kk in range(KT1):
                        nc.tensor.matmul(
                            out=ps[:], lhsT=xs[:, kk, t * P:(t + 1) * P],
                            rhs=w1_t[:, kk, :],
                            start=(kk == 0), stop=(kk == KT1 - 1))
                    st = stagep.tile([P, H], BF16, tag="Ast")
                    nc.scalar.activation(st[:], ps[:],
                                         mybir.ActivationFunctionType.Copy,
                                         bias=0.0, scale=dinv_tt[:, g:g + 1])
                    nc.sync.dma_start(xw1s[g * P:(g + 1) * P, :], st[:])

            # ---------------- phase B: L1 aggregation per window
            for w in range(n_win):
                cw = chunks_w[w]
                co = chunk_off[w]
                sw = sldp.tile([P, max(chunks_w) * P], FP8, tag="s1")
                nc.sync.dma_start(sw[:, :cw * P],
                                  soh[:, co * P:(co + cw) * P])
                ps = psB.tile([P, H], F32, space="PSUM")
                for c in range(cw):
                    g = gthp.tile([P, H], BF16, tag="g1")
                    nc.gpsimd.indirect_dma_start(
                        out=g[:], out_offset=None, in_=xw1s[:],
                        in_offset=bass.IndirectOffsetOnAxis(
                            ap=idx_t[:, co + c:co + c + 1], axis=0))
                    nc.tensor.matmul(out=ps[:], lhsT=sw[:, c * P:(c + 1) * P],
                                     rhs=g[:], start=(c == 0), stop=(c == cw - 1))
                t1 = epip.tile([P, H], F32, tag="b_t1")
                nc.vector.tensor_scalar(out=t1[:], in0=ps[:],
                                        scalar1=dinv_ot[:, w:w + 1], scalar2=None,
                                        op0=mybir.AluOpType.mult)
                nc.vector.tensor_add(t1[:], t1[:], b1_t[:])
                hb = epip.tile([P, H], BF16, tag="b_h")
                nc.vector.tensor_scalar(out=hb[:], in0=t1[:], scalar1=0.0,
                                        scalar2=None, op0=mybir.AluOpType.max)
                nc.sync.dma_start(h_loc[w * P:(w + 1) * P, :], hb[:])

            # ---------------- phase C: hw = (h @ W2p) * dinv
            for t in range(n_win):
                ps = psC.tile([P, Cpad], F32, space="PSUM")
                for kk in range(KT2):
                    ht = stagep.tile([P, P], BF16, tag="hT")
                    nc.sync.dma_start_transpose(
                        ht[:], h_loc[t * P:(t + 1) * P, kk * P:(kk + 1) * P])
                    nc.tensor.matmul(out=ps[:], lhsT=ht[:], rhs=w2_t[:, kk, :],
                                     start=(kk == 0), stop=(kk == KT2 - 1))
                st = stagep.tile([P, Cpad], BF16, tag="Cst")
                nc.scalar.activation(st[:], ps[:],
                                     mybir.ActivationFunctionType.Copy,
                                     bias=0.0, scale=dinv_ot[:, t:t + 1])
                nc.sync.dma_start(hw_loc[t * P:(t + 1) * P, :], st[:])

            # ---------------- phase D: AllGather hw -> hw2s
            nc.gpsimd.collective_compute(
                "AllGather", mybir.AluOpType.bypass,
                replica_groups=[list(range(N_CORES))],
                ins=[hw_loc[:].opt()], outs=[hw2s[:].opt()])

            # ---------------- phase E: L2 aggregation + log_softmax
            CC = L["C"]
            for w in range(n_win):
                cw = chunks_w[w]
                co = chunk_off[w]
                sw = sldp.tile([P, max(chunks_w) * P], FP8, tag="s2")
                nc.sync.dma_start(sw[:, :cw * P],
                                  soh[:, co * P:(co + cw) * P])
                ps = psE.tile([P, Cpad], F32, space="PSUM")
                for c in range(cw):
                    g = gthp.tile([P, Cpad], BF16, tag="g2")
                    nc.gpsimd.indirect_dma_start(
                        out=g[:], out_offset=None, in_=hw2s[:],
                        in_offset=bass.IndirectOffsetOnAxis(
                            ap=idx_t[:, co + c:co + c + 1], axis=0))
                    nc.tensor.matmul(out=ps[:], lhsT=sw[:, c * P:(c + 1) * P],
                                     rhs=g[:], start=(c == 0), stop=(c == cw - 1))
                z = epip.tile([P, Cpad], F32, tag="e_z")
                nc.vector.tensor_scalar(out=z[:], in0=ps[:],
                                        scalar1=dinv_ot[:, w:w + 1], scalar2=None,
                                        op0=mybir.AluOpType.mult)
                nc.vector.tensor_add(z[:], z[:], b2_t[:])
                mneg = epip.tile([P, 1], F32, tag="e_m")
                nc.vector.tensor_reduce(out=mneg[:], in_=z[:, :CC],
                                        axis=mybir.AxisListType.X,
                                        op=mybir.AluOpType.max, negate=True)
                ex = epip.tile([P, CC], F32, tag="e_ex")
                ssum = epip.tile([P, 1], F32, tag="e_s")
                nc.scalar.activation(ex[:], z[:, :CC],
                                     mybir.ActivationFunctionType.Exp,
                                     bias=mneg[:], scale=1.0, accum_out=ssum[:])
                lns = epip.tile([P, 1], F32, tag="e_l")
                nc.scalar.activation(lns[:], ssum[:],
                                     mybir.ActivationFunctionType.Ln)
                cc_t = epip.tile([P, 1], F32, tag="e_c")
                nc.vector.tensor_scalar(out=cc_t[:], in0=lns[:], scalar1=mneg[:],
                                        scalar2=None, op0=mybir.AluOpType.subtract)
                zo = epip.tile([P, Cpad], F32, tag="e_o")
                nc.vector.tensor_scalar(out=zo[:], in0=z[:],
                                        scalar1=cc_t[:], scalar2=None,
                                        op0=mybir.AluOpType.subtract)
                nc.sync.dma_start(out[w * P:(w + 1) * P, :], zo[:])

    nc.compile()
    return nc


# ------------------------------------------------------------------ interface

def _get_runner(L):
    key = tuple(sorted((k, v if not isinstance(v, list) else tuple(v))
                       for k, v in L.items()))
    if key in _RUN_CACHE:
        return _RUN_CACHE[key]
    nc = _build(L)
    from concourse.bass_utils import run_bass_kernel_spmd

    def run(in_maps):
        return run_bass_kernel_spmd(nc, in_maps, core_ids=list(range(N_CORES)))
    _RUN_CACHE[key] = (nc, run)
    return nc, run


def kernel(x, edge_index, W1, b1, W2, b2):
    x = np.asarray(x)
    edge_index = np.asarray(edge_index)
    in_dtype = edge_index.dtype
    L, in_maps = _preprocess(x, edge_index, np.asarray(W1), np.asarray(b1),
                             np.asarray(W2), np.asarray(b2))
    nc, run = _get_runner(L)
    res = run(in_maps)
    N, C, shard = L["N"], L["C"], L["shard"]
    parts = [res.results[k]["out"][:, :C] for k in range(N_CORES)]
    return np.concatenate(parts, axis=0)[:N].astype(np.float32)
